# revision 57
# baseline (speedup 1.0000x reference)
"""NodeSetAttention TRN2 Bass kernel.

Shapes (hardcoded): B=4, S=2048, D=256, H=8, HD=32, E=65536.
Sharding: 8 cores, core c -> (batch b = c//2, query half qh = c%2).
Each core computes the full transformer block for its 1024 query rows,
recomputing K/V for the full 2048 keys of its batch (cheap).

Mask+edge-bias handling: host precomputes
    expA[b][k, q] = adj[b, q, k] * exp(clip(edge_bias2d[q, k], -5, 5))
(fp16, transposed to [k, q]).  Device computes
    attn_unnorm = exp(qk_T - 3) * expA        (the -3 keeps fp16 in range;
it cancels in the softmax normalization).  Denominators come from a ones
column appended to V (v_aug[k, 33]); PV matmul accumulates both ctx and
denominator in PSUM.
"""

import math
from contextlib import ExitStack

import numpy as np
import ml_dtypes

BF16NP = ml_dtypes.bfloat16

import concourse.bass as bass
import concourse.bacc as bacc
import concourse.mybir as mybir
import concourse.tile as tile
from concourse.bass_utils import run_bass_kernel_spmd
from concourse.dve_ops import AFFINE_THEN_ADD
from concourse.masks import make_identity

B, S, D, H, E = 4, 2048, 256, 8, 65536
HD = D // H          # 32
QH = S // 2          # 1024 query rows per core
NCORES = 8
DT = S // 128        # 16 s-tiles over full S
QT = QH // 128       # 8 q-tiles per core
F32 = mybir.dt.float32
F32R = mybir.dt.float32r
FP16 = mybir.dt.float16
BF16 = mybir.dt.bfloat16
AF = mybir.ActivationFunctionType
OP = mybir.AluOpType
EXP_SHIFT = -3.0     # exp(logit + EXP_SHIFT); cancels in normalization
A16 = 1024.0 / math.log(2.0)         # Schraudolph scale, folded into wk
S1MAGIC = 15360.0 - 45.0 + A16 * EXP_SHIFT  # fp16 bias - minimax corr + shift
I16 = mybir.dt.int16
# kt tiles using the fused DVE Schraudolph route (i16 saturating add of
# A16-scaled logits + A16*log-mask, bitcast to fp16); rest use ACT exp
DVE_KTS = (0, 3, 5, 8, 11, 14)
# pool muls early in the head so all et tiles are ready when the next
# head's PV chains (which need every kt) start
POOL_MUL_KTS = (1, 4, 7, 10)


def _emit(ctx: ExitStack, tc: tile.TileContext):
    nc = tc.nc
    xb = nc.dram_tensor("xb", [S, D], F32, kind="ExternalInput").ap()
    expa = nc.dram_tensor("expa", [S, QH], FP16, kind="ExternalInput").ap()
    wqt = nc.dram_tensor("wqt", [D, D], BF16, kind="ExternalInput").ap()
    wkt = nc.dram_tensor("wkt", [D, D], BF16, kind="ExternalInput").ap()
    wvt = nc.dram_tensor("wvt", [D, D], BF16, kind="ExternalInput").ap()
    wot = nc.dram_tensor("wot", [D, D], F32R, kind="ExternalInput").ap()
    w1t = nc.dram_tensor("w1t", [D, 4 * D], F32R, kind="ExternalInput").ap()
    w2t = nc.dram_tensor("w2t", [4 * D, D], F32R, kind="ExternalInput").ap()
    # blob cols: 0 bq(scaled) 1 bk 2 bo 3 b2 4 ln1g 5 ln1b 6 ln2g 7 ln2b
    blob = nc.dram_tensor("blob", [D, 8], F32, kind="ExternalInput").ap()
    # 64-row chunked layout for q/k biases: [64, chunk(4), (bq, bk)]
    blob64 = nc.dram_tensor("blob64", [64, 8], F32, kind="ExternalInput").ap()
    b1v = nc.dram_tensor("b1v", [4 * D, 1], F32, kind="ExternalInput").ap()
    bvr = nc.dram_tensor("bvr", [1, D], F32R, kind="ExternalInput").ap()
    bor = nc.dram_tensor("bor", [1, D], F32R, kind="ExternalInput").ap()
    b2r = nc.dram_tensor("b2r", [1, D], F32R, kind="ExternalInput").ap()
    onesr = nc.dram_tensor("onesr", [1, 128], F32R, kind="ExternalInput").ap()
    out = nc.dram_tensor("out", [QH, D], F32, kind="ExternalOutput").ap()

    # ---------------- pools ----------------
    consts = ctx.enter_context(tc.tile_pool(name="consts", bufs=1))
    main = ctx.enter_context(tc.tile_pool(name="main", bufs=1))
    tmp = ctx.enter_context(tc.tile_pool(name="tmp", bufs=8))
    small = ctx.enter_context(tc.tile_pool(name="small", bufs=24))

    # ---------------- const tiles ----------------
    wq_sb = consts.tile([128, 2, D], BF16, tag="wq")
    wk_sb = consts.tile([128, 2, D], BF16, tag="wk")
    wv_sb = consts.tile([128, 2, D], BF16, tag="wv")
    wo_sb = consts.tile([128, 2, D], F32R, tag="wo")
    blob_sb = consts.tile([128, 2, 8], F32, tag="blob")
    blob64_sb = consts.tile([64, 4, 2], F32, tag="blob64")
    bv_sb = consts.tile([128, D], F32R, tag="bv")
    bo_sb = consts.tile([1, D], F32R, tag="bo")
    b2_sb = consts.tile([1, D], F32R, tag="b2")
    w1_sb = consts.tile([128, 2, 4 * D], F32R, tag="w1")
    w2_sb = consts.tile([128, 8, D], F32R, tag="w2")
    b1_sb = consts.tile([128, 8, 1], F32, tag="b1")
    ident = consts.tile([128, 128], F32, tag="ident")
    ones_sb = consts.tile([128, 128], F32R, tag="ones")
    ones5 = consts.tile([1, 512], F32, tag="ones5")
    eps_sb = consts.tile([128, 1], F32, tag="eps")
    shift_sb = consts.tile([128, 1], F32, tag="shift")

    def bcol(i):  # [128,1] per-partition column from blob, d-tile dt
        return lambda dt: blob_sb[:, dt, i : i + 1]

    bo_c, b2_c = bcol(2), bcol(3)
    g1_c, bt1_c, g2_c, bt2_c = bcol(4), bcol(5), bcol(6), bcol(7)

    ctx_sb = main.tile([128, QT, D], F32, tag="ctx")
    xres_sb = main.tile([128, QT, D], F32, tag="xres")

    # LN stats: returns (rstd, nmr) [128,1] tiles; normalize done by caller
    def ln_stats(x_ap):
        st = small.tile([128, nc.vector.BN_STATS_DIM], F32, tag="bnst")
        mv = small.tile([128, nc.vector.BN_AGGR_DIM], F32, tag="bnmv")
        nc.vector.bn_stats(out=st, in_=x_ap)
        nc.vector.bn_aggr(out=mv, in_=st)
        rstd = small.tile([128, 1], F32, tag="rstd")
        nmr = small.tile([128, 1], F32, tag="nmr")
        nc.scalar.activation(out=rstd, in_=mv[:, 1:2], func=AF.Sqrt, bias=eps_sb)
        nc.vector.reciprocal(out=rstd, in_=rstd)
        nc.vector.tensor_scalar(
            out=nmr, in0=mv[:, 0:1], scalar1=rstd, scalar2=-1.0, op0=OP.mult, op1=OP.mult
        )
        return rstd, nmr

    # PSUM->SBUF copy with per-partition scale/bias, alternating ACT/DVE
    def scale_bias_copy(out_ap, in_ap, scale_ap, bias_ap, use_act):
        if use_act:
            nc.scalar.activation(
                out=out_ap, in_=in_ap, func=AF.Identity,
                bias=bias_ap if bias_ap is not None else 0.0,
                scale=scale_ap if scale_ap is not None else 1.0,
            )
        elif scale_ap is None:
            nc.vector.tensor_scalar(
                out=out_ap, in0=in_ap, scalar1=bias_ap, scalar2=None, op0=OP.add
            )
        else:
            nc.vector.tensor_scalar(
                out=out_ap, in0=in_ap, scalar1=scale_ap, scalar2=bias_ap,
                op0=OP.mult, op1=OP.add,
            )

    with tc.tile_pool(name="attn", bufs=1) as attn:
        # head-pair layout: [hd + 32*(h%2), ht, s] so matmul base partition
        # offsets stay in {0, 32} (PE requires base in {0,32,64})
        qT_t = [attn.tile([64, QH], BF16, tag=f"qT{m}", name=f"qT{m}") for m in range(4)]
        kT_t = [attn.tile([64, S], BF16, tag=f"kT{m}", name=f"kT{m}") for m in range(4)]
        vaug = attn.tile([128, DT, H, HD + 1], FP16, tag="vaug")  # [s, h, f+1]
        nxT = attn.tile([128, 2, S], BF16, tag="nxT")
        expa_g = [
            attn.tile([128, 4, QH], FP16, tag=f"expa{g}", name=f"expa{g}")
            for g in range(4)
        ]
        expa_t = [expa_g[k // 4][:, k % 4, :] for k in range(DT)]
        expa_r = expa.rearrange("(g j p) q -> g p j q", p=128, j=4)

        with tc.tile_pool(name="prolog", bufs=1) as prolog, tc.tile_pool(
            name="ps_pro", bufs=4, space="PSUM"
        ) as ps_pro:
            xb_g = [
                prolog.tile([128, 4, D], F32, tag=f"xb{g}", name=f"xb{g}")
                for g in range(4)
            ]
            xb_t = [xb_g[s // 4][:, s % 4, :] for s in range(DT)]
            xb_r = xb.rearrange("(g j p) d -> g p j d", p=128, j=4)
            for g in range(4):
                nc.sync.dma_start(out=xb_g[g], in_=xb_r[g])
            for g in range(2):
                nc.gpsimd.tensor_copy(
                    out=xres_sb[:, g * 4 : g * 4 + 4, :], in_=xb_g[g]
                )
            # small consts next (they gate the earliest LN copies)
            nc.sync.dma_start(out=blob_sb, in_=blob.rearrange("(t p) i -> p t i", p=128))
            nc.sync.dma_start(out=blob64_sb, in_=blob64.rearrange("p (t i) -> p t i", t=4))
            nc.sync.dma_start(out=bv_sb[0:1, :], in_=bvr)
            nc.sync.dma_start(out=bo_sb, in_=bor)
            nc.sync.dma_start(out=b2_sb, in_=b2r)
            nc.sync.dma_start(out=ones_sb[0:1, :], in_=onesr)
            make_identity(nc, ident[:])
            nc.vector.memset(eps_sb, 1e-5)
            nc.gpsimd.memset(ones5, 1.0)
            nc.vector.memset(shift_sb, EXP_SHIFT)
            # q/k weights split in halves so mt0 projections start early
            wqr = wqt.rearrange("(t p) m -> p t m", p=128)
            wkr = wkt.rearrange("(t p) m -> p t m", p=128)
            nc.sync.dma_start(out=wq_sb[:, :, 0:128], in_=wqr[:, :, 0:128])
            nc.sync.dma_start(out=wk_sb[:, :, 0:128], in_=wkr[:, :, 0:128])
            # ones column of vaug (disjoint from the V writes)
            nc.gpsimd.memset(vaug[:, :, :, HD : HD + 1], 1.0)

            # ---- LN1 + transpose to nxT (fold ln1 g/b after transpose);
            # 4 transposes packed per psum tile -> one 512-wide copy
            last_rstd = None
            for sg in range(DT // 4):
                nxs = []
                for j in range(4):
                    st_i = sg * 4 + j
                    nx = tmp.tile([128, D], F32, tag="nx")
                    rstd, nmr = ln_stats(xb_t[st_i])
                    last_rstd = rstd
                    scale_bias_copy(nx, xb_t[st_i], rstd, nmr, use_act=(st_i % 2 == 1))
                    nxs.append(nx)
                for dt in range(2):
                    pst = ps_pro.tile([128, 512], F32, tag="pp")
                    for j in range(4):
                        nc.tensor.transpose(
                            pst[:, j * 128 : j * 128 + 128],
                            nxs[j][:, dt * 128 : dt * 128 + 128],
                            ident[:],
                        )
                    # plain copy: ln1 g/b are folded into wq/wk/wv on host
                    if dt == 0:
                        nc.scalar.activation(
                            out=nxT[:, dt, sg * 512 : sg * 512 + 512],
                            in_=pst[:], func=AF.Copy,
                        )
                    else:
                        nc.vector.tensor_copy(
                            out=nxT[:, dt, sg * 512 : sg * 512 + 512], in_=pst[:]
                        )
            # expa g0 early (right after the mt0 weight halves) so the exp
            # stream starts ~13us in; later weights gate only later heads
            nc.sync.dma_start(out=expa_g[0], in_=expa_r[0])
            nc.sync.dma_start(out=expa_g[1], in_=expa_r[1])
            nc.sync.dma_start(out=wq_sb[:, :, 128:256], in_=wqr[:, :, 128:256])
            nc.sync.dma_start(out=wk_sb[:, :, 128:256], in_=wkr[:, :, 128:256])
            nc.sync.dma_start(out=wv_sb, in_=wvt.rearrange("(t p) m -> p t m", p=128))
            nc.sync.dma_start(out=expa_g[2], in_=expa_r[2])
            nc.sync.dma_start(out=wo_sb, in_=wot.rearrange("(t p) m -> p t m", p=128))
            nc.sync.dma_start(out=expa_g[3], in_=expa_r[3])
            nc.sync.dma_start(out=w1_sb, in_=w1t.rearrange("(t p) m -> p t m", p=128))
            nc.sync.dma_start(out=w2_sb, in_=w2t.rearrange("(t p) m -> p t m", p=128))
            nc.sync.dma_start(out=b1_sb, in_=b1v.rearrange("(t p) i -> p t i", p=128))

            # only the mt0 projections (heads 0/1) in the prolog: anything
            # more would queue ahead of h0's QK on PE and stall on late DMAs
            ncopy = 0

            def proj_piece(dst_t, w_sb, bias_i, mt, nb, pool, tag="pp"):
                nonlocal ncopy
                ps = pool.tile([64, 512], F32, tag=tag)
                for dt in range(2):
                    nc.tensor.matmul(
                        ps[:],
                        (w_sb[:, dt, mt * 64 : mt * 64 + 64]),
                        (nxT[:, dt, nb * 512 : nb * 512 + 512]),
                        start=(dt == 0),
                        stop=(dt == 1),
                    )
                ncopy += 1
                scale_bias_copy(
                    dst_t[mt][:, nb * 512 : nb * 512 + 512], ps[:],
                    None, blob64_sb[:, mt, bias_i : bias_i + 1], ncopy % 2 == 0,
                )

            for nb in range(2):
                proj_piece(qT_t, wq_sb, 0, 0, nb, ps_pro)
            for nb in range(4):
                proj_piece(kT_t, wk_sb, 1, 0, nb, ps_pro)
        # prolog + ps_pro released (xb freed); mt1-3 projections and the V
        # projection run as extras inside heads 0-5

        # ---- attention: software-pipelined heads, kt-interleaved PV ----
        with tc.tile_pool(name="ps_sc", bufs=3, space="PSUM") as ps_sc, tc.tile_pool(
            name="ps_pv", bufs=2, space="PSUM"
        ) as ps_pv, tc.tile_pool(name="pvsb", bufs=2) as pvsb, tc.tile_pool(
            name="expt", bufs=32
        ) as expt_pool:

            def v_piece(st_i):
                ps = ps_pv.tile([128, 256], F32, tag="pvh")
                for dt in range(2):
                    nc.tensor.matmul(
                        ps[:],
                        (nxT[:, dt, st_i * 128 : st_i * 128 + 128]),
                        (wv_sb[:, dt, :]),
                        start=(dt == 0),
                        stop=False,
                    )
                nc.tensor.matmul(
                    ps[:], (ones_sb[0:1, :]), (bv_sb[0:1, :]), start=False, stop=True
                )
                if st_i % 2 == 0:
                    nc.scalar.activation(
                        out=vaug[:, st_i, :, 0:HD],
                        in_=ps.rearrange("p (h f) -> p h f", h=H),
                        func=AF.Copy,
                    )
                else:
                    nc.vector.tensor_copy(
                        out=vaug[:, st_i, :, 0:HD],
                        in_=ps.rearrange("p (h f) -> p h f", h=H),
                    )

            # V pieces 2/kt over h0 kt8..15 (wv lands ~18us; all pieces must
            # land before h0's PV chains start at h1 kt1); mt1-3 projections
            # spread over heads 1/2/4 kts 8..13 (after chains+finalize)
            extras = {}
            for st_i in range(DT):
                extras.setdefault((0, 8 + st_i // 2), []).append(
                    lambda s=st_i: v_piece(s)
                )
            for hsrc, mt in ((1, 1), (2, 2), (4, 3)):
                pieces = [
                    lambda nb=nb, mt=mt: proj_piece(qT_t, wq_sb, 0, mt, nb, ps_pv, "pvh")
                    for nb in range(2)
                ] + [
                    lambda nb=nb, mt=mt: proj_piece(kT_t, wk_sb, 1, mt, nb, ps_pv, "pvh")
                    for nb in range(4)
                ]
                for i, th in enumerate(pieces):
                    extras.setdefault((hsrc, 8 + i), []).append(th)

            def qk_step(h, kt):
                hp = (h % 2) * HD
                ht = h // 2
                ps = ps_sc.tile([128, QH], F32, tag="sc")
                for qb in range(QH // 512):
                    nc.tensor.matmul(
                        ps[:, qb * 512 : qb * 512 + 512],
                        (kT_t[ht][hp : hp + HD, kt * 128 : kt * 128 + 128]),
                        (qT_t[ht][hp : hp + HD, qb * 512 : qb * 512 + 512]),
                    )
                ex = expt_pool.tile([128, QH], FP16, tag="expt", name=f"ex{h}_{kt}")
                if kt in DVE_KTS:
                    # fused exp+mask: i16 = sat_rne(A16*s + S1MAGIC + la[k,q]),
                    # bitcast fp16 (masked la=-65504 saturates to -0.0)
                    nc.vector._custom_dve(
                        AFFINE_THEN_ADD, out=ex.bitcast(I16), in0=ps[:],
                        in1=expa_t[kt], s0=1.0, s1=S1MAGIC,
                    )
                else:
                    nc.scalar.activation(
                        out=ex, in_=ps[:], func=AF.Exp, bias=shift_sb,
                        scale=1.0 / A16,
                    )
                    eng = nc.gpsimd if kt in POOL_MUL_KTS else nc.vector
                    eng.tensor_mul(ex, ex, expa_t[kt])
                return ex

            def pv_chain(h, et, qt, pvh):
                for kt in range(DT):
                    nc.tensor.matmul(
                        pvh[:, qt, :],
                        et[kt][:, qt * 128 : qt * 128 + 128],
                        vaug[:, kt, h, :],
                        start=(kt == 0),
                        stop=(kt == DT - 1),
                    )

            def pv_reduce(h, pvh):
                # one PSUM->SBUF copy + one batched reciprocal per head
                pvs = pvsb.tile([128, QT, HD + 1], F32, tag="pvs")
                nc.vector.tensor_copy(out=pvs, in_=pvh)
                dn8 = small.tile([128, QT], F32, tag="dn8")
                nc.vector.reciprocal(out=dn8, in_=pvs[:, :, HD : HD + 1])
                return pvs, dn8

            def pv_norm(h, pvs, dn8, qt):
                # per-qt normalize on the idle Pool engine (SBUF-only there)
                nc.gpsimd.tensor_scalar(
                    out=ctx_sb[:, qt, h * HD : h * HD + HD],
                    in0=pvs[:, qt, 0:HD],
                    scalar1=dn8[:, qt : qt + 1],
                    scalar2=None,
                    op0=OP.mult,
                )

            prev = None
            pvh_prev = None
            fin_prev = None
            for h in range(H):
                et = []
                for kt in range(DT):
                    et.append(qk_step(h, kt))
                    for th in extras.get((h, kt), []):
                        th()
                    # prev head's PV chains in kts 4..11 (muls done; releases
                    # free expt slots before this head's tail allocations)
                    if prev is not None and 1 <= kt <= 4:
                        if kt == 1:
                            pvh_prev = ps_pv.tile([128, QT, HD + 1], F32, tag="pvh")
                        pv_chain(h - 1, prev, (kt - 1) * 2, pvh_prev)
                        pv_chain(h - 1, prev, (kt - 1) * 2 + 1, pvh_prev)
                    if prev is not None and kt == 5:
                        fin_prev = pv_reduce(h - 1, pvh_prev)
                    # normalizes spread 2/kt so the Pool wait-queue (depth 4)
                    # never blocks the muls queued behind them
                    if prev is not None and 6 <= kt <= 9:
                        pv_norm(h - 1, *fin_prev, (kt - 6) * 2)
                        pv_norm(h - 1, *fin_prev, (kt - 6) * 2 + 1)
                prev = et
            pvh_prev = ps_pv.tile([128, QT, HD + 1], F32, tag="pvh")
            for qt in range(QT):
                pv_chain(H - 1, prev, qt, pvh_prev)
            fin_prev = pv_reduce(H - 1, pvh_prev)
            for qt in range(QT):
                pv_norm(H - 1, *fin_prev, qt)

    # ---------------- post-attention (attn pools released) ----------------
    with tc.tile_pool(name="mlp", bufs=1) as mlp, tc.tile_pool(
        name="ps_mlp", bufs=4, space="PSUM"
    ) as ps_mlp, tc.tile_pool(name="ps_mlp2", bufs=2, space="PSUM") as ps_mlp2:
        NB = QH // 512  # 2
        ctxT = [mlp.tile([128, 2, 512], F32R, tag=f"ctxT{b}", name=f"ctxT{b}") for b in range(NB)]
        yT = [mlp.tile([128, 2, 512], F32, tag=f"yT{b}", name=f"yT{b}") for b in range(NB)]
        y_sb = mlp.tile([128, QT, D], F32, tag="y")
        n2T = [mlp.tile([128, 2, 512], F32R, tag=f"n2T{b}", name=f"n2T{b}") for b in range(NB)]
        hT = [mlp.tile([128, 8, 512], F32R, tag=f"hT{b}", name=f"hT{b}") for b in range(NB)]
        o2T = [mlp.tile([128, 2, 512], F32, tag=f"o2T{b}", name=f"o2T{b}") for b in range(NB)]

        # stage-major: each stage runs for both qbs back-to-back so unlike
        # ACT functions don't thrash the activation table and each engine
        # gets long runs of like work
        for qb in range(NB):
            # transpose ctx -> ctxT[qb]: all 8 blocks into one wide psum tile
            pst = ps_mlp2.tile([128, 1024], F32, tag="pm2")
            for dt in range(2):
                for qq in range(4):
                    qt = qb * 4 + qq
                    nc.tensor.transpose(
                        pst[:, dt * 512 + qq * 128 : dt * 512 + qq * 128 + 128],
                        ctx_sb[:, qt, dt * 128 : dt * 128 + 128],
                        ident[:],
                    )
            if qb == 0:
                nc.scalar.activation(out=ctxT[qb][:], in_=pst[:], func=AF.Copy)
            else:
                nc.vector.tensor_copy(out=ctxT[qb][:], in_=pst[:])
        for qb in range(NB):
            # O-projection: yT = wo @ ctxT + bo (bias via ones-row matmul;
            # both mt chunks land in one wide psum tile -> one plain copy)
            ps = ps_mlp2.tile([128, 1024], F32, tag="pm2")
            for mt in range(2):
                for dt in range(2):
                    nc.tensor.matmul(
                        ps[:, mt * 512 : mt * 512 + 512],
                        (wo_sb[:, dt, mt * 128 : mt * 128 + 128]),
                        (ctxT[qb][:, dt, :]),
                        start=(dt == 0),
                        stop=False,
                    )
                nc.tensor.matmul(
                    ps[:, mt * 512 : mt * 512 + 512],
                    (bo_sb[:, mt * 128 : mt * 128 + 128]),
                    (ones5.bitcast(F32R)),
                    start=False,
                    stop=True,
                )
            if qb == 0:
                nc.vector.tensor_copy(out=yT[qb][:], in_=ps[:])
            else:
                nc.scalar.activation(out=yT[qb][:], in_=ps[:], func=AF.Copy)
        for qb in range(NB):
            # transpose back (2 qt x 2 mt per psum tile) + residual
            for qp in range(2):
                qt0 = qb * 4 + qp * 2
                pst = ps_mlp.tile([128, 512], F32, tag="pm")
                for j in range(2):
                    for mt in range(2):
                        nc.tensor.transpose(
                            pst[:, j * 256 + mt * 128 : j * 256 + mt * 128 + 128],
                            yT[qb][:, mt, (qp * 2 + j) * 128 : (qp * 2 + j) * 128 + 128],
                            ident[:],
                        )
                nc.vector.tensor_tensor(
                    out=y_sb[:, qt0 : qt0 + 2, :].rearrange("p a b -> p (a b)"),
                    in0=pst[:],
                    in1=xres_sb[:, qt0 : qt0 + 2, :].rearrange("p a b -> p (a b)"),
                    op=OP.add,
                )
        # LN2 for all 8 qt (sqrts batched -> one act-table context);
        # normalize copies go to the idle Pool engine (SBUF->SBUF)
        n2s = []
        stats = []
        for qt in range(QT):
            stats.append(ln_stats(y_sb[:, qt, :]))
        for qt in range(QT):
            n2 = tmp.tile([128, D], F32, tag="nx")
            rstd, nmr = stats[qt]
            nc.gpsimd.tensor_scalar(
                out=n2, in0=y_sb[:, qt, :], scalar1=rstd, scalar2=nmr,
                op0=OP.mult, op1=OP.add,
            )
            n2s.append(n2)
        for qb in range(NB):
            # transpose -> n2T, one wide psum tile + one plain copy
            # (ln2 g/b are folded into w1/b1 on the host)
            pst = ps_mlp2.tile([128, 1024], F32, tag="pm2")
            for dt in range(2):
                for qq in range(4):
                    nc.tensor.transpose(
                        pst[:, dt * 512 + qq * 128 : dt * 512 + qq * 128 + 128],
                        n2s[qb * 4 + qq][:, dt * 128 : dt * 128 + 128],
                        ident[:],
                    )
            if qb == 0:
                nc.scalar.activation(out=n2T[qb][:], in_=pst[:], func=AF.Copy)
            else:
                nc.vector.tensor_copy(out=n2T[qb][:], in_=pst[:])

        for qb in range(NB):
            # MLP: hT = gelu(w1 @ n2T + b1)
            for mt in range(8):
                ps = ps_mlp.tile([128, 512], F32, tag="pm")
                for dt in range(2):
                    nc.tensor.matmul(
                        ps[:],
                        (w1_sb[:, dt, mt * 128 : mt * 128 + 128]),
                        (n2T[qb][:, dt, :]),
                        start=(dt == 0),
                        stop=(dt == 1),
                    )
                nc.scalar.activation(
                    out=hT[qb][:, mt, :],
                    in_=ps[:],
                    func=AF.Gelu,
                    bias=b1_sb[:, mt, :],
                )
            # o2T = w2 @ hT + b2 (bias via ones-row matmul; one wide copy)
            ps2 = ps_mlp2.tile([128, 1024], F32, tag="pm2")
            for mt in range(2):
                for dt in range(8):
                    nc.tensor.matmul(
                        ps2[:, mt * 512 : mt * 512 + 512],
                        (w2_sb[:, dt, mt * 128 : mt * 128 + 128]),
                        (hT[qb][:, dt, :]),
                        start=(dt == 0),
                        stop=False,
                    )
                nc.tensor.matmul(
                    ps2[:, mt * 512 : mt * 512 + 512],
                    (b2_sb[:, mt * 128 : mt * 128 + 128]),
                    (ones5.bitcast(F32R)),
                    start=False,
                    stop=True,
                )
            if qb == 0:
                nc.scalar.activation(out=o2T[qb][:], in_=ps2[:], func=AF.Copy)
            else:
                nc.vector.tensor_copy(out=o2T[qb][:], in_=ps2[:])
            # transpose back + final residual into ctx_sb; DMA out per qb
            for qp in range(2):
                qt0 = qb * 4 + qp * 2
                pst = ps_mlp.tile([128, 512], F32, tag="pm")
                for j in range(2):
                    for mt in range(2):
                        nc.tensor.transpose(
                            pst[:, j * 256 + mt * 128 : j * 256 + mt * 128 + 128],
                            o2T[qb][:, mt, (qp * 2 + j) * 128 : (qp * 2 + j) * 128 + 128],
                            ident[:],
                        )
                nc.vector.tensor_tensor(
                    out=ctx_sb[:, qt0 : qt0 + 2, :].rearrange("p a b -> p (a b)"),
                    in0=pst[:],
                    in1=y_sb[:, qt0 : qt0 + 2, :].rearrange("p a b -> p (a b)"),
                    op=OP.add,
                )
                nc.sync.dma_start(
                    out=out.rearrange("(t p) d -> p t d", p=128)[:, qt0 : qt0 + 2, :],
                    in_=ctx_sb[:, qt0 : qt0 + 2, :],
                )


_NC_CACHE = {}


def _get_nc():
    if "nc" not in _NC_CACHE:
        nc = bacc.Bacc("TRN2", target_bir_lowering=False, debug=False)
        with tile.TileContext(nc) as tc:
            with ExitStack() as ctx:
                _emit(ctx, tc)
        nc.compile()
        _NC_CACHE["nc"] = nc
    return _NC_CACHE["nc"]


def _prep_common(inputs):
    f = lambda k: np.asarray(inputs[k], np.float32)
    sc = 1.0 / math.sqrt(HD)
    wq, wk, wv, wo = f("wq"), f("wk"), f("wv"), f("wo")
    w1, w2 = f("w1"), f("w2")
    g1, b1c = f("ln1_g"), f("ln1_b")
    g2, b2c = f("ln2_g"), f("ln2_b")
    # fold LN affine params into the following projections:
    #   (z*g + b) @ W^T + bias  ==  z @ (W*g)^T + (bias + W @ b)
    bq = f("bq") + wq @ b1c
    bk = f("bk") + wk @ b1c
    bv = f("bv") + wv @ b1c
    b1 = f("b1") + w1 @ b2c
    wq = wq * g1[None, :]
    wk = wk * g1[None, :]
    wv = wv * g1[None, :]
    w1 = w1 * g2[None, :]
    blob = np.stack(
        [bq * sc, bk * A16, f("bo"), f("b2"), g1, b1c, g2, b2c], axis=1
    )  # [256, 8]
    bq_s = (bq * sc).reshape(4, 64).T  # [64, 4]
    bk_s = (bk * A16).reshape(4, 64).T
    blob64 = np.stack([bq_s, bk_s], axis=2).reshape(64, 8)  # [64, chunk, item]
    return {
        "blob64": np.ascontiguousarray(blob64),
        "onesr": np.ones((1, 128), np.float32),
        "wqt": (np.ascontiguousarray(wq.T) * sc).astype(BF16NP),
        "wkt": (np.ascontiguousarray(wk.T) * A16).astype(BF16NP),
        "wvt": np.ascontiguousarray(wv.T).astype(BF16NP),
        "wot": np.ascontiguousarray(wo.T),
        "w1t": np.ascontiguousarray(w1.T),
        "w2t": np.ascontiguousarray(w2.T),
        "blob": np.ascontiguousarray(blob),
        "b1v": np.ascontiguousarray(b1.reshape(4 * D, 1)),
        "bvr": np.ascontiguousarray(bv.reshape(1, D)),
        "bor": np.ascontiguousarray(f("bo").reshape(1, D)),
        "b2r": np.ascontiguousarray(f("b2").reshape(1, D)),
    }


def _run(inputs, trace=False):
    x = np.asarray(inputs["x"], np.float32)
    adj = np.asarray(inputs["adj_mask"]).astype(bool)
    ea = np.asarray(inputs["edge_attr"], np.float32).reshape(-1)
    ei = np.asarray(inputs["edge_index"]).astype(np.int64)

    bias2d = np.zeros((S, S), np.float32)
    bias2d[ei[0], ei[1]] = np.clip(ea, -5.0, 5.0)
    expb = np.exp(bias2d)  # [q, k] layout

    common = _prep_common(inputs)
    in_maps = []
    for c in range(NCORES):
        b, qh = c // 2, c % 2
        r0 = qh * QH
        xc = x[b]
        if qh == 1:  # rotate halves so our queries are rows [0, 1024)
            xc = np.concatenate([xc[QH:], xc[:QH]], axis=0)
        # expa rows (k) must follow the SAME rotated key order as xc
        adj_c = adj[b, r0 : r0 + QH, :]
        bias_c = bias2d[r0 : r0 + QH, :]
        ea_c = expb[r0 : r0 + QH, :] * adj_c  # [q, k], k in orig order
        la_c = np.where(adj_c, A16 * bias_c, -65504.0)  # [q, k] log-domain
        if qh == 1:
            ea_c = np.concatenate([ea_c[:, QH:], ea_c[:, :QH]], axis=1)
            la_c = np.concatenate([la_c[:, QH:], la_c[:, :QH]], axis=1)
        ea_c = np.ascontiguousarray(ea_c.T).astype(np.float16)  # [k, q]
        la_c = la_c.T.astype(np.float16)
        for kt in DVE_KTS:  # fused-route rows carry log-domain masks
            ea_c[kt * 128 : kt * 128 + 128, :] = la_c[kt * 128 : kt * 128 + 128, :]
        in_maps.append(
            {"xb": np.ascontiguousarray(xc), "expa": ea_c, **common}
        )

    nc = _get_nc()
    res = run_bass_kernel_spmd(
        nc, in_maps, core_ids=list(range(NCORES)), trace=trace
    )
    outs = [res.results[c]["out"] for c in range(NCORES)]
    y = np.stack(
        [np.concatenate([outs[2 * b], outs[2 * b + 1]], axis=0) for b in range(B)],
        axis=0,
    )
    return y, res


def kernel(**inputs) -> np.ndarray:
    y, _ = _run(inputs, trace=False)
    return y



# revision 58
# speedup vs baseline: 1.0291x; 1.0291x over previous
"""NodeSetAttention TRN2 Bass kernel.

Shapes (hardcoded): B=4, S=2048, D=256, H=8, HD=32, E=65536.
Sharding: 8 cores, core c -> (batch b = c//2, query half qh = c%2).
Each core computes the full transformer block for its 1024 query rows,
recomputing K/V for the full 2048 keys of its batch (cheap).

Mask+edge-bias handling: host precomputes
    expA[b][k, q] = adj[b, q, k] * exp(clip(edge_bias2d[q, k], -5, 5))
(fp16, transposed to [k, q]).  Device computes
    attn_unnorm = exp(qk_T - 3) * expA        (the -3 keeps fp16 in range;
it cancels in the softmax normalization).  Denominators come from a ones
column appended to V (v_aug[k, 33]); PV matmul accumulates both ctx and
denominator in PSUM.
"""

import math
from contextlib import ExitStack

import numpy as np
import ml_dtypes

BF16NP = ml_dtypes.bfloat16

import concourse.bass as bass
import concourse.bacc as bacc
import concourse.mybir as mybir
import concourse.tile as tile
from concourse.bass_utils import run_bass_kernel_spmd
from concourse.dve_ops import AFFINE_THEN_ADD
from concourse.masks import make_identity

B, S, D, H, E = 4, 2048, 256, 8, 65536
HD = D // H          # 32
QH = S // 2          # 1024 query rows per core
NCORES = 8
DT = S // 128        # 16 s-tiles over full S
QT = QH // 128       # 8 q-tiles per core
F32 = mybir.dt.float32
F32R = mybir.dt.float32r
FP16 = mybir.dt.float16
BF16 = mybir.dt.bfloat16
AF = mybir.ActivationFunctionType
OP = mybir.AluOpType
EXP_SHIFT = -3.0     # exp(logit + EXP_SHIFT); cancels in normalization
A16 = 1024.0 / math.log(2.0)         # Schraudolph scale, folded into wk
S1MAGIC = 15360.0 - 45.0 + A16 * EXP_SHIFT  # fp16 bias - minimax corr + shift
I16 = mybir.dt.int16
# kt tiles using the fused DVE Schraudolph route (i16 saturating add of
# A16-scaled logits + A16*log-mask, bitcast to fp16); rest use ACT exp
DVE_KTS = (0, 3, 5, 8, 11, 14)
# pool muls early in the head so all et tiles are ready when the next
# head's PV chains (which need every kt) start
POOL_MUL_KTS = (1, 4, 7, 10)


def _emit(ctx: ExitStack, tc: tile.TileContext):
    nc = tc.nc
    xb = nc.dram_tensor("xb", [S, D], F32, kind="ExternalInput").ap()
    expa = nc.dram_tensor("expa", [S, QH], FP16, kind="ExternalInput").ap()
    wqt = nc.dram_tensor("wqt", [D, D], BF16, kind="ExternalInput").ap()
    wkt = nc.dram_tensor("wkt", [D, D], BF16, kind="ExternalInput").ap()
    wvt = nc.dram_tensor("wvt", [D, D], BF16, kind="ExternalInput").ap()
    wot = nc.dram_tensor("wot", [D, D], F32R, kind="ExternalInput").ap()
    w1t = nc.dram_tensor("w1t", [D, 4 * D], F32R, kind="ExternalInput").ap()
    w2t = nc.dram_tensor("w2t", [4 * D, D], F32R, kind="ExternalInput").ap()
    # blob cols: 0 bq(scaled) 1 bk 2 bo 3 b2 4 ln1g 5 ln1b 6 ln2g 7 ln2b
    blob = nc.dram_tensor("blob", [D, 8], F32, kind="ExternalInput").ap()
    # 64-row chunked layout for q/k biases: [64, chunk(4), (bq, bk)]
    blob64 = nc.dram_tensor("blob64", [64, 8], F32, kind="ExternalInput").ap()
    b1v = nc.dram_tensor("b1v", [4 * D, 1], F32, kind="ExternalInput").ap()
    bvr = nc.dram_tensor("bvr", [1, D], F32R, kind="ExternalInput").ap()
    bor = nc.dram_tensor("bor", [1, D], F32R, kind="ExternalInput").ap()
    b2r = nc.dram_tensor("b2r", [1, D], F32R, kind="ExternalInput").ap()
    onesr = nc.dram_tensor("onesr", [1, 128], F32R, kind="ExternalInput").ap()
    out = nc.dram_tensor("out", [QH, D], F32, kind="ExternalOutput").ap()

    # ---------------- pools ----------------
    consts = ctx.enter_context(tc.tile_pool(name="consts", bufs=1))
    main = ctx.enter_context(tc.tile_pool(name="main", bufs=1))
    tmp = ctx.enter_context(tc.tile_pool(name="tmp", bufs=8))
    small = ctx.enter_context(tc.tile_pool(name="small", bufs=24))

    # ---------------- const tiles ----------------
    wq_sb = consts.tile([128, 2, D], BF16, tag="wq")
    wk_sb = consts.tile([128, 2, D], BF16, tag="wk")
    wv_sb = consts.tile([128, 2, D], BF16, tag="wv")
    wo_sb = consts.tile([128, 2, D], F32R, tag="wo")
    blob_sb = consts.tile([128, 2, 8], F32, tag="blob")
    blob64_sb = consts.tile([64, 4, 2], F32, tag="blob64")
    bv_sb = consts.tile([128, D], F32R, tag="bv")
    bo_sb = consts.tile([1, D], F32R, tag="bo")
    b2_sb = consts.tile([1, D], F32R, tag="b2")
    w1_sb = consts.tile([128, 2, 4 * D], F32R, tag="w1")
    w2_sb = consts.tile([128, 8, D], F32R, tag="w2")
    b1_sb = consts.tile([128, 8, 1], F32, tag="b1")
    ident = consts.tile([128, 128], F32, tag="ident")
    ones_sb = consts.tile([128, 128], F32R, tag="ones")
    ones5 = consts.tile([1, 512], F32, tag="ones5")
    eps_sb = consts.tile([128, 1], F32, tag="eps")
    shift_sb = consts.tile([128, 1], F32, tag="shift")

    def bcol(i):  # [128,1] per-partition column from blob, d-tile dt
        return lambda dt: blob_sb[:, dt, i : i + 1]

    bo_c, b2_c = bcol(2), bcol(3)
    g1_c, bt1_c, g2_c, bt2_c = bcol(4), bcol(5), bcol(6), bcol(7)

    ctx_sb = main.tile([128, QT, D], F32, tag="ctx")
    xres_sb = main.tile([128, QT, D], F32, tag="xres")

    # LN stats: returns (rstd, nmr) [128,1] tiles; normalize done by caller
    def ln_stats(x_ap):
        st = small.tile([128, nc.vector.BN_STATS_DIM], F32, tag="bnst")
        mv = small.tile([128, nc.vector.BN_AGGR_DIM], F32, tag="bnmv")
        nc.vector.bn_stats(out=st, in_=x_ap)
        nc.vector.bn_aggr(out=mv, in_=st)
        rstd = small.tile([128, 1], F32, tag="rstd")
        nmr = small.tile([128, 1], F32, tag="nmr")
        nc.scalar.activation(out=rstd, in_=mv[:, 1:2], func=AF.Sqrt, bias=eps_sb)
        nc.vector.reciprocal(out=rstd, in_=rstd)
        nc.vector.tensor_scalar(
            out=nmr, in0=mv[:, 0:1], scalar1=rstd, scalar2=-1.0, op0=OP.mult, op1=OP.mult
        )
        return rstd, nmr

    # PSUM->SBUF copy with per-partition scale/bias, alternating ACT/DVE
    def scale_bias_copy(out_ap, in_ap, scale_ap, bias_ap, use_act):
        if use_act:
            nc.scalar.activation(
                out=out_ap, in_=in_ap, func=AF.Identity,
                bias=bias_ap if bias_ap is not None else 0.0,
                scale=scale_ap if scale_ap is not None else 1.0,
            )
        elif scale_ap is None:
            nc.vector.tensor_scalar(
                out=out_ap, in0=in_ap, scalar1=bias_ap, scalar2=None, op0=OP.add
            )
        else:
            nc.vector.tensor_scalar(
                out=out_ap, in0=in_ap, scalar1=scale_ap, scalar2=bias_ap,
                op0=OP.mult, op1=OP.add,
            )

    with tc.tile_pool(name="attn", bufs=1) as attn:
        # head-pair layout: [hd + 32*(h%2), ht, s] so matmul base partition
        # offsets stay in {0, 32} (PE requires base in {0,32,64})
        qT_t = [attn.tile([64, QH], BF16, tag=f"qT{m}", name=f"qT{m}") for m in range(4)]
        kT_t = [attn.tile([64, S], BF16, tag=f"kT{m}", name=f"kT{m}") for m in range(4)]
        vaug = attn.tile([128, DT, H, HD + 1], FP16, tag="vaug")  # [s, h, f+1]
        nxT = attn.tile([128, 2, S], BF16, tag="nxT")
        expa_g = [
            attn.tile([128, 4, QH], FP16, tag=f"expa{g}", name=f"expa{g}")
            for g in range(4)
        ]
        expa_t = [expa_g[k // 4][:, k % 4, :] for k in range(DT)]
        expa_r = expa.rearrange("(g j p) q -> g p j q", p=128, j=4)

        with tc.tile_pool(name="prolog", bufs=1) as prolog, tc.tile_pool(
            name="ps_pro", bufs=4, space="PSUM"
        ) as ps_pro:
            xb_g = [
                prolog.tile([128, 4, D], F32, tag=f"xb{g}", name=f"xb{g}")
                for g in range(4)
            ]
            xb_t = [xb_g[s // 4][:, s % 4, :] for s in range(DT)]
            xb_r = xb.rearrange("(g j p) d -> g p j d", p=128, j=4)
            for g in range(4):
                nc.sync.dma_start(out=xb_g[g], in_=xb_r[g])
            for g in range(2):
                nc.gpsimd.tensor_copy(
                    out=xres_sb[:, g * 4 : g * 4 + 4, :], in_=xb_g[g]
                )
            # small consts next (they gate the earliest LN copies)
            nc.sync.dma_start(out=blob_sb, in_=blob.rearrange("(t p) i -> p t i", p=128))
            nc.sync.dma_start(out=blob64_sb, in_=blob64.rearrange("p (t i) -> p t i", t=4))
            nc.sync.dma_start(out=bv_sb[0:1, :], in_=bvr)
            nc.sync.dma_start(out=bo_sb, in_=bor)
            nc.sync.dma_start(out=b2_sb, in_=b2r)
            nc.sync.dma_start(out=ones_sb[0:1, :], in_=onesr)
            make_identity(nc, ident[:])
            nc.vector.memset(eps_sb, 1e-5)
            nc.gpsimd.memset(ones5, 1.0)
            nc.vector.memset(shift_sb, EXP_SHIFT)
            # q/k weights split in halves so mt0 projections start early
            wqr = wqt.rearrange("(t p) m -> p t m", p=128)
            wkr = wkt.rearrange("(t p) m -> p t m", p=128)
            nc.sync.dma_start(out=wq_sb[:, :, 0:128], in_=wqr[:, :, 0:128])
            nc.sync.dma_start(out=wk_sb[:, :, 0:128], in_=wkr[:, :, 0:128])
            # ones column of vaug (disjoint from the V writes)
            nc.gpsimd.memset(vaug[:, :, :, HD : HD + 1], 1.0)

            # ---- LN1 + transpose to nxT (fold ln1 g/b after transpose);
            # 4 transposes packed per psum tile -> one 512-wide copy
            last_rstd = None
            for sg in range(DT // 4):
                nxs = []
                for j in range(4):
                    st_i = sg * 4 + j
                    nx = tmp.tile([128, D], F32, tag="nx")
                    rstd, nmr = ln_stats(xb_t[st_i])
                    last_rstd = rstd
                    scale_bias_copy(nx, xb_t[st_i], rstd, nmr, use_act=(st_i % 2 == 1))
                    nxs.append(nx)
                for dt in range(2):
                    pst = ps_pro.tile([128, 512], F32, tag="pp")
                    for j in range(4):
                        nc.tensor.transpose(
                            pst[:, j * 128 : j * 128 + 128],
                            nxs[j][:, dt * 128 : dt * 128 + 128],
                            ident[:],
                        )
                    # plain copy: ln1 g/b are folded into wq/wk/wv on host
                    if dt == 0:
                        nc.scalar.activation(
                            out=nxT[:, dt, sg * 512 : sg * 512 + 512],
                            in_=pst[:], func=AF.Copy,
                        )
                    else:
                        nc.vector.tensor_copy(
                            out=nxT[:, dt, sg * 512 : sg * 512 + 512], in_=pst[:]
                        )
            # expa g0 early (right after the mt0 weight halves) so the exp
            # stream starts ~13us in; later weights gate only later heads
            nc.sync.dma_start(out=expa_g[0], in_=expa_r[0])
            nc.sync.dma_start(out=expa_g[1], in_=expa_r[1])
            nc.sync.dma_start(out=wq_sb[:, :, 128:256], in_=wqr[:, :, 128:256])
            nc.sync.dma_start(out=wk_sb[:, :, 128:256], in_=wkr[:, :, 128:256])
            nc.sync.dma_start(out=wv_sb, in_=wvt.rearrange("(t p) m -> p t m", p=128))
            nc.sync.dma_start(out=expa_g[2], in_=expa_r[2])
            nc.sync.dma_start(out=wo_sb, in_=wot.rearrange("(t p) m -> p t m", p=128))
            nc.sync.dma_start(out=expa_g[3], in_=expa_r[3])
            nc.sync.dma_start(out=w1_sb, in_=w1t.rearrange("(t p) m -> p t m", p=128))
            nc.sync.dma_start(out=w2_sb, in_=w2t.rearrange("(t p) m -> p t m", p=128))
            nc.sync.dma_start(out=b1_sb, in_=b1v.rearrange("(t p) i -> p t i", p=128))

            # only the mt0 projections (heads 0/1) in the prolog: anything
            # more would queue ahead of h0's QK on PE and stall on late DMAs
            ncopy = 0

            def proj_piece(dst_t, w_sb, bias_i, mt, nb, pool, tag="pp"):
                nonlocal ncopy
                ps = pool.tile([64, 512], F32, tag=tag)
                for dt in range(2):
                    nc.tensor.matmul(
                        ps[:],
                        (w_sb[:, dt, mt * 64 : mt * 64 + 64]),
                        (nxT[:, dt, nb * 512 : nb * 512 + 512]),
                        start=(dt == 0),
                        stop=(dt == 1),
                    )
                ncopy += 1
                scale_bias_copy(
                    dst_t[mt][:, nb * 512 : nb * 512 + 512], ps[:],
                    None, blob64_sb[:, mt, bias_i : bias_i + 1], ncopy % 2 == 0,
                )

            for nb in range(2):
                proj_piece(qT_t, wq_sb, 0, 0, nb, ps_pro)
            for nb in range(4):
                proj_piece(kT_t, wk_sb, 1, 0, nb, ps_pro)
        # prolog + ps_pro released (xb freed); mt1-3 projections and the V
        # projection run as extras inside heads 0-5

        # ---- attention: software-pipelined heads, kt-interleaved PV ----
        with tc.tile_pool(name="ps_sc", bufs=3, space="PSUM") as ps_sc, tc.tile_pool(
            name="ps_pv", bufs=2, space="PSUM"
        ) as ps_pv, tc.tile_pool(name="pvsb", bufs=2) as pvsb, tc.tile_pool(
            name="expt", bufs=32
        ) as expt_pool:

            def v_piece(st_i):
                ps = ps_pv.tile([128, 256], F32, tag="pvh")
                for dt in range(2):
                    nc.tensor.matmul(
                        ps[:],
                        (nxT[:, dt, st_i * 128 : st_i * 128 + 128]),
                        (wv_sb[:, dt, :]),
                        start=(dt == 0),
                        stop=False,
                    )
                nc.tensor.matmul(
                    ps[:], (ones_sb[0:1, :]), (bv_sb[0:1, :]), start=False, stop=True
                )
                if st_i % 2 == 0:
                    nc.scalar.activation(
                        out=vaug[:, st_i, :, 0:HD],
                        in_=ps.rearrange("p (h f) -> p h f", h=H),
                        func=AF.Copy,
                    )
                else:
                    nc.vector.tensor_copy(
                        out=vaug[:, st_i, :, 0:HD],
                        in_=ps.rearrange("p (h f) -> p h f", h=H),
                    )

            # V pieces 2/kt over h0 kt8..15 (wv lands ~18us; all pieces must
            # land before h0's PV chains start at h1 kt1); mt1-3 projections
            # spread over heads 1/2/4 kts 8..13 (after chains+finalize)
            extras = {}
            for st_i in range(DT):
                extras.setdefault((0, 8 + st_i // 2), []).append(
                    lambda s=st_i: v_piece(s)
                )
            for hsrc, mt in ((1, 1), (2, 2), (4, 3)):
                pieces = [
                    lambda nb=nb, mt=mt: proj_piece(qT_t, wq_sb, 0, mt, nb, ps_pv, "pvh")
                    for nb in range(2)
                ] + [
                    lambda nb=nb, mt=mt: proj_piece(kT_t, wk_sb, 1, mt, nb, ps_pv, "pvh")
                    for nb in range(4)
                ]
                for i, th in enumerate(pieces):
                    extras.setdefault((hsrc, 8 + i), []).append(th)

            def qk_step(h, kt):
                hp = (h % 2) * HD
                ht = h // 2
                ps = ps_sc.tile([128, QH], F32, tag="sc")
                for qb in range(QH // 512):
                    nc.tensor.matmul(
                        ps[:, qb * 512 : qb * 512 + 512],
                        (kT_t[ht][hp : hp + HD, kt * 128 : kt * 128 + 128]),
                        (qT_t[ht][hp : hp + HD, qb * 512 : qb * 512 + 512]),
                    )
                ex = expt_pool.tile([128, QH], FP16, tag="expt", name=f"ex{h}_{kt}")
                if kt in DVE_KTS:
                    # fused exp+mask: i16 = sat_rne(A16*s + S1MAGIC + la[k,q]),
                    # bitcast fp16 (masked la=-65504 saturates to -0.0)
                    nc.vector._custom_dve(
                        AFFINE_THEN_ADD, out=ex.bitcast(I16), in0=ps[:],
                        in1=expa_t[kt], s0=1.0, s1=S1MAGIC,
                    )
                else:
                    nc.scalar.activation(
                        out=ex, in_=ps[:], func=AF.Exp, bias=shift_sb,
                        scale=1.0 / A16,
                    )
                    eng = nc.gpsimd if kt in POOL_MUL_KTS else nc.vector
                    eng.tensor_mul(ex, ex, expa_t[kt])
                return ex

            def pv_chain(h, et, qt, pvh):
                for kt in range(DT):
                    nc.tensor.matmul(
                        pvh[:, qt, :],
                        et[kt][:, qt * 128 : qt * 128 + 128],
                        vaug[:, kt, h, :],
                        start=(kt == 0),
                        stop=(kt == DT - 1),
                    )

            def pv_reduce(h, pvh):
                # one PSUM->SBUF copy + one batched reciprocal per head
                pvs = pvsb.tile([128, QT, HD + 1], F32, tag="pvs")
                nc.vector.tensor_copy(out=pvs, in_=pvh)
                dn8 = small.tile([128, QT], F32, tag="dn8")
                nc.vector.reciprocal(out=dn8, in_=pvs[:, :, HD : HD + 1])
                return pvs, dn8

            def pv_norm(h, pvs, dn8, qt):
                # per-qt normalize on the idle Pool engine (SBUF-only there)
                nc.gpsimd.tensor_scalar(
                    out=ctx_sb[:, qt, h * HD : h * HD + HD],
                    in0=pvs[:, qt, 0:HD],
                    scalar1=dn8[:, qt : qt + 1],
                    scalar2=None,
                    op0=OP.mult,
                )

            prev = None
            pvh_prev = None
            fin_prev = None
            for h in range(H):
                et = []
                for kt in range(DT):
                    et.append(qk_step(h, kt))
                    for th in extras.get((h, kt), []):
                        th()
                    # prev head's PV chains in kts 4..11 (muls done; releases
                    # free expt slots before this head's tail allocations)
                    if prev is not None and 1 <= kt <= 4:
                        if kt == 1:
                            pvh_prev = ps_pv.tile([128, QT, HD + 1], F32, tag="pvh")
                        pv_chain(h - 1, prev, (kt - 1) * 2, pvh_prev)
                        pv_chain(h - 1, prev, (kt - 1) * 2 + 1, pvh_prev)
                    if prev is not None and kt == 5:
                        fin_prev = pv_reduce(h - 1, pvh_prev)
                    # normalizes spread 2/kt so the Pool wait-queue (depth 4)
                    # never blocks the muls queued behind them
                    if prev is not None and 6 <= kt <= 9:
                        pv_norm(h - 1, *fin_prev, (kt - 6) * 2)
                        pv_norm(h - 1, *fin_prev, (kt - 6) * 2 + 1)
                prev = et
            pvh_prev = ps_pv.tile([128, QT, HD + 1], F32, tag="pvh")
            for qt in range(QT):
                pv_chain(H - 1, prev, qt, pvh_prev)
            fin_prev = pv_reduce(H - 1, pvh_prev)
            for qt in range(QT):
                pv_norm(H - 1, *fin_prev, qt)

    # ---------------- post-attention (attn pools released) ----------------
    with tc.tile_pool(name="mlp", bufs=1) as mlp, tc.tile_pool(
        name="ps_mlp", bufs=6, space="PSUM"
    ) as ps_mlp:
        NB = QH // 512  # 2
        ctxT = [mlp.tile([128, 2, 512], F32R, tag=f"ctxT{b}", name=f"ctxT{b}") for b in range(NB)]
        yT = [mlp.tile([128, 2, 512], F32, tag=f"yT{b}", name=f"yT{b}") for b in range(NB)]
        y_sb = mlp.tile([128, QT, D], F32, tag="y")
        n2T = [mlp.tile([128, 2, 512], F32R, tag=f"n2T{b}", name=f"n2T{b}") for b in range(NB)]
        hT = [mlp.tile([128, 8, 512], F32R, tag=f"hT{b}", name=f"hT{b}") for b in range(NB)]
        o2T = [mlp.tile([128, 2, 512], F32, tag=f"o2T{b}", name=f"o2T{b}") for b in range(NB)]

        # stage-major: each stage runs for both qbs back-to-back so unlike
        # ACT functions don't thrash the activation table and each engine
        # gets long runs of like work
        for qb in range(NB):
            # transpose ctx -> ctxT[qb]: 4 qt per psum tile, per dt
            for dt in range(2):
                pst = ps_mlp.tile([128, 512], F32, tag="pm")
                for qq in range(4):
                    qt = qb * 4 + qq
                    nc.tensor.transpose(
                        pst[:, qq * 128 : qq * 128 + 128],
                        ctx_sb[:, qt, dt * 128 : dt * 128 + 128],
                        ident[:],
                    )
                if dt == 0:
                    nc.scalar.activation(out=ctxT[qb][:, dt, :], in_=pst[:], func=AF.Copy)
                else:
                    nc.vector.tensor_copy(out=ctxT[qb][:, dt, :], in_=pst[:])
        for qb in range(NB):
            # O-projection: yT = wo @ ctxT + bo
            for mt in range(2):
                ps = ps_mlp.tile([128, 512], F32, tag="pm")
                for dt in range(2):
                    nc.tensor.matmul(
                        ps[:],
                        (wo_sb[:, dt, mt * 128 : mt * 128 + 128]),
                        (ctxT[qb][:, dt, :]),
                        start=(dt == 0),
                        stop=(dt == 1),
                    )
                scale_bias_copy(yT[qb][:, mt, :], ps[:], None, bo_c(mt), use_act=(mt == 0))
        for qb in range(NB):
            # transpose back (2 qt x 2 mt per psum tile) + residual
            for qp in range(2):
                qt0 = qb * 4 + qp * 2
                pst = ps_mlp.tile([128, 512], F32, tag="pm")
                for j in range(2):
                    for mt in range(2):
                        nc.tensor.transpose(
                            pst[:, j * 256 + mt * 128 : j * 256 + mt * 128 + 128],
                            yT[qb][:, mt, (qp * 2 + j) * 128 : (qp * 2 + j) * 128 + 128],
                            ident[:],
                        )
                nc.vector.tensor_tensor(
                    out=y_sb[:, qt0 : qt0 + 2, :].rearrange("p a b -> p (a b)"),
                    in0=pst[:],
                    in1=xres_sb[:, qt0 : qt0 + 2, :].rearrange("p a b -> p (a b)"),
                    op=OP.add,
                )
        # LN2 for all 8 qt (sqrts batched -> one act-table context);
        # normalize copies go to the idle Pool engine (SBUF->SBUF)
        n2s = []
        stats = []
        for qt in range(QT):
            stats.append(ln_stats(y_sb[:, qt, :]))
        for qt in range(QT):
            n2 = tmp.tile([128, D], F32, tag="nx")
            rstd, nmr = stats[qt]
            nc.gpsimd.tensor_scalar(
                out=n2, in0=y_sb[:, qt, :], scalar1=rstd, scalar2=nmr,
                op0=OP.mult, op1=OP.add,
            )
            n2s.append(n2)
        for qb in range(NB):
            # transpose -> n2T (4 qt per psum tile, per dt); plain copies:
            # ln2 g/b are folded into w1/b1 on the host
            for dt in range(2):
                pst = ps_mlp.tile([128, 512], F32, tag="pm")
                for qq in range(4):
                    nc.tensor.transpose(
                        pst[:, qq * 128 : qq * 128 + 128],
                        n2s[qb * 4 + qq][:, dt * 128 : dt * 128 + 128],
                        ident[:],
                    )
                if dt == 0:
                    nc.scalar.activation(out=n2T[qb][:, dt, :], in_=pst[:], func=AF.Copy)
                else:
                    nc.vector.tensor_copy(out=n2T[qb][:, dt, :], in_=pst[:])

        for qb in range(NB):
            # MLP: hT = gelu(w1 @ n2T + b1)
            for mt in range(8):
                ps = ps_mlp.tile([128, 512], F32, tag="pm")
                for dt in range(2):
                    nc.tensor.matmul(
                        ps[:],
                        (w1_sb[:, dt, mt * 128 : mt * 128 + 128]),
                        (n2T[qb][:, dt, :]),
                        start=(dt == 0),
                        stop=(dt == 1),
                    )
                nc.scalar.activation(
                    out=hT[qb][:, mt, :],
                    in_=ps[:],
                    func=AF.Gelu,
                    bias=b1_sb[:, mt, :],
                )
            # o2T = w2 @ hT + b2
            for mt in range(2):
                ps = ps_mlp.tile([128, 512], F32, tag="pm")
                for dt in range(8):
                    nc.tensor.matmul(
                        ps[:],
                        (w2_sb[:, dt, mt * 128 : mt * 128 + 128]),
                        (hT[qb][:, dt, :]),
                        start=(dt == 0),
                        stop=(dt == 7),
                    )
                scale_bias_copy(o2T[qb][:, mt, :], ps[:], None, b2_c(mt), use_act=(mt == 1))
            # transpose back + final residual into ctx_sb; DMA out per qb
            for qp in range(2):
                qt0 = qb * 4 + qp * 2
                pst = ps_mlp.tile([128, 512], F32, tag="pm")
                for j in range(2):
                    for mt in range(2):
                        nc.tensor.transpose(
                            pst[:, j * 256 + mt * 128 : j * 256 + mt * 128 + 128],
                            o2T[qb][:, mt, (qp * 2 + j) * 128 : (qp * 2 + j) * 128 + 128],
                            ident[:],
                        )
                nc.vector.tensor_tensor(
                    out=ctx_sb[:, qt0 : qt0 + 2, :].rearrange("p a b -> p (a b)"),
                    in0=pst[:],
                    in1=y_sb[:, qt0 : qt0 + 2, :].rearrange("p a b -> p (a b)"),
                    op=OP.add,
                )
                nc.sync.dma_start(
                    out=out.rearrange("(t p) d -> p t d", p=128)[:, qt0 : qt0 + 2, :],
                    in_=ctx_sb[:, qt0 : qt0 + 2, :],
                )


_NC_CACHE = {}


def _get_nc():
    if "nc" not in _NC_CACHE:
        nc = bacc.Bacc("TRN2", target_bir_lowering=False, debug=False)
        with tile.TileContext(nc) as tc:
            with ExitStack() as ctx:
                _emit(ctx, tc)
        nc.compile()
        _NC_CACHE["nc"] = nc
    return _NC_CACHE["nc"]


def _prep_common(inputs):
    f = lambda k: np.asarray(inputs[k], np.float32)
    sc = 1.0 / math.sqrt(HD)
    wq, wk, wv, wo = f("wq"), f("wk"), f("wv"), f("wo")
    w1, w2 = f("w1"), f("w2")
    g1, b1c = f("ln1_g"), f("ln1_b")
    g2, b2c = f("ln2_g"), f("ln2_b")
    # fold LN affine params into the following projections:
    #   (z*g + b) @ W^T + bias  ==  z @ (W*g)^T + (bias + W @ b)
    bq = f("bq") + wq @ b1c
    bk = f("bk") + wk @ b1c
    bv = f("bv") + wv @ b1c
    b1 = f("b1") + w1 @ b2c
    wq = wq * g1[None, :]
    wk = wk * g1[None, :]
    wv = wv * g1[None, :]
    w1 = w1 * g2[None, :]
    blob = np.stack(
        [bq * sc, bk * A16, f("bo"), f("b2"), g1, b1c, g2, b2c], axis=1
    )  # [256, 8]
    bq_s = (bq * sc).reshape(4, 64).T  # [64, 4]
    bk_s = (bk * A16).reshape(4, 64).T
    blob64 = np.stack([bq_s, bk_s], axis=2).reshape(64, 8)  # [64, chunk, item]
    return {
        "blob64": np.ascontiguousarray(blob64),
        "onesr": np.ones((1, 128), np.float32),
        "wqt": (np.ascontiguousarray(wq.T) * sc).astype(BF16NP),
        "wkt": (np.ascontiguousarray(wk.T) * A16).astype(BF16NP),
        "wvt": np.ascontiguousarray(wv.T).astype(BF16NP),
        "wot": np.ascontiguousarray(wo.T),
        "w1t": np.ascontiguousarray(w1.T),
        "w2t": np.ascontiguousarray(w2.T),
        "blob": np.ascontiguousarray(blob),
        "b1v": np.ascontiguousarray(b1.reshape(4 * D, 1)),
        "bvr": np.ascontiguousarray(bv.reshape(1, D)),
        "bor": np.ascontiguousarray(f("bo").reshape(1, D)),
        "b2r": np.ascontiguousarray(f("b2").reshape(1, D)),
    }


def _run(inputs, trace=False):
    x = np.asarray(inputs["x"], np.float32)
    adj = np.asarray(inputs["adj_mask"]).astype(bool)
    ea = np.asarray(inputs["edge_attr"], np.float32).reshape(-1)
    ei = np.asarray(inputs["edge_index"]).astype(np.int64)

    bias2d = np.zeros((S, S), np.float32)
    bias2d[ei[0], ei[1]] = np.clip(ea, -5.0, 5.0)
    expb = np.exp(bias2d)  # [q, k] layout

    common = _prep_common(inputs)
    in_maps = []
    for c in range(NCORES):
        b, qh = c // 2, c % 2
        r0 = qh * QH
        xc = x[b]
        if qh == 1:  # rotate halves so our queries are rows [0, 1024)
            xc = np.concatenate([xc[QH:], xc[:QH]], axis=0)
        # expa rows (k) must follow the SAME rotated key order as xc
        adj_c = adj[b, r0 : r0 + QH, :]
        bias_c = bias2d[r0 : r0 + QH, :]
        ea_c = expb[r0 : r0 + QH, :] * adj_c  # [q, k], k in orig order
        la_c = np.where(adj_c, A16 * bias_c, -65504.0)  # [q, k] log-domain
        if qh == 1:
            ea_c = np.concatenate([ea_c[:, QH:], ea_c[:, :QH]], axis=1)
            la_c = np.concatenate([la_c[:, QH:], la_c[:, :QH]], axis=1)
        ea_c = np.ascontiguousarray(ea_c.T).astype(np.float16)  # [k, q]
        la_c = la_c.T.astype(np.float16)
        for kt in DVE_KTS:  # fused-route rows carry log-domain masks
            ea_c[kt * 128 : kt * 128 + 128, :] = la_c[kt * 128 : kt * 128 + 128, :]
        in_maps.append(
            {"xb": np.ascontiguousarray(xc), "expa": ea_c, **common}
        )

    nc = _get_nc()
    res = run_bass_kernel_spmd(
        nc, in_maps, core_ids=list(range(NCORES)), trace=trace
    )
    outs = [res.results[c]["out"] for c in range(NCORES)]
    y = np.stack(
        [np.concatenate([outs[2 * b], outs[2 * b + 1]], axis=0) for b in range(B)],
        axis=0,
    )
    return y, res


def kernel(**inputs) -> np.ndarray:
    y, _ = _run(inputs, trace=False)
    return y

